# revision 27
# baseline (speedup 1.0000x reference)
"""AnomalyTransformer forward on 8 TRN2 NeuronCores.

Sharding: collective-free. Core c computes batch element b=c//2 end-to-end
(pairs duplicate the per-token pipeline), but each core materializes/writes
only 4 of the 8 attention heads' series/prior tensors. Odd cores see the
head-blocks of Wq/Wk/Wv/Wsig (cols) and Wo (rows) pre-permuted host-side so
the SPMD graph always outputs "heads 0-3" of its own view.

Precision: residual stream, LayerNorms, softmax sums, series/prior outputs in
fp32; Q/K/V projections, attention and FFN matmuls in bf16 (weights stored
bf16 in HBM, halving weight reads); Wo and the final projection in float32r
(tf32-like full-rate fp32). Softmax skips max-subtraction (|scores| <~ 1 by
construction). The softmax recip is folded per-partition after exp (series
output) and after the unnormalized attn^T matmul, so series^T is obtained by
SBUF->SBUF DMA-transposes of the bf16 exp tiles instead of a second matmul
and exp pass.

All weight/constant arrays are pre-tiled host-side so every DMA is one
contiguous 128-partition block.
"""
import sys

sys.path.insert(0, '/opt/trn_rl_repo')

import math
import numpy as np

B, L, CIN, D, H, NL, DFF = 4, 512, 38, 512, 8, 3, 2048
E = D // H           # 64
HH = H // 2          # 4 output heads per core
NC = 4               # 128-token chunks per L
ND = 4               # 128-row chunks per D
NK = DFF // 128      # 16 dff chunks
P = 128
LN3 = math.log(3.0)
INV_SQRT_2PI = 1.0 / math.sqrt(2.0 * math.pi)
LN_INV_SQRT_2PI = -0.5 * math.log(2.0 * math.pi)

_CACHE = {}


# ---------------------------------------------------------------------------
# walrus on this image rejects instructions with more than one sync wait.
# Split excess waits onto nofuse nops on the same engine.
# ---------------------------------------------------------------------------

def _patch_drain_wait_split():
    from concourse import tile as tile_mod
    import bass_rust

    if getattr(tile_mod.TileContext._drain_and_barrier, '_waitfix', False):
        return

    def patched(self, tick_clock, wait_clock):
        from concourse.vector_clock import ScopedClock

        nc = self.nc
        drain_inst = nc.sync.drain()
        wait_clock.add_sem_waits(
            drain_inst.ins, ScopedClock({None: tick_clock.global_clock})
        )
        si = drain_inst.ins.sync_info
        waits = list(si.on_wait)
        if len(waits) > 1:
            si.on_wait = waits[:1]
            drain_inst.ins.sync_info = si
            for w in waits[1:]:
                nop = nc.sync.nop(nofuse=True, hint="drain_wait_split")
                nop.ins.sync_info = bass_rust.SyncInfo(on_wait=[w], on_update=[])

        nc.all_engine_barrier()
        assert self.sems is not None
        popped = nc._tile_sem_poison_stack.pop()
        assert popped is self._sem_poison
        nc.clear_and_free_semaphores(list(self.sems.allocated().values()))
        nc.all_engine_barrier()

    patched._waitfix = True
    tile_mod.TileContext._drain_and_barrier = patched


def _fix_multi_waits(nc):
    """Post-pass: any instruction carrying >1 sem waits gets the extra waits
    hoisted onto nofuse nops inserted just before it (same engine)."""
    from concourse import mybir

    n = 0
    for f in nc.m.functions:
        for bb in f.blocks:
            insts = list(bb.instructions)
            out = []
            changed = False
            for ins in insts:
                si = ins.sync_info
                if si is not None and len(si.on_wait) > 1:
                    waits = list(si.on_wait)
                    for w in waits[:-1]:
                        n += 1
                        nop = mybir.InstNoOp(
                            name=f"waitfix-{n}",
                            engine=ins.engine,
                            bass_nofuse=True,
                            sync_info=mybir.SyncInfo(on_wait=[w], on_update=[]),
                        )
                        out.append(nop)
                    si.on_wait = waits[-1:]
                    ins.sync_info = si
                    changed = True
                out.append(ins)
            if changed:
                bb.instructions = out
    return n


# ---------------------------------------------------------------------------
# Device graph
# ---------------------------------------------------------------------------

def _build_nc():
    import concourse.bass as bass
    import concourse.mybir as mybir
    from concourse.tile import TileContext
    from concourse.masks import make_identity

    f32 = mybir.dt.float32
    f32r = mybir.dt.float32r
    bf16 = mybir.dt.bfloat16
    nc = bass.Bass()

    def param(name, shape, dt=None):
        return nc.declare_dram_parameter(
            name, list(shape), dt or f32, isOutput=False
        )

    # pre-tiled host layouts (see _stage_core)
    xcatT_d = param("xcatT", (3 * CIN, L), f32r)
    tokw_d = param("tokw", (3 * CIN, D), f32r)
    pos_d = param("pos", (P, NC, D))
    dist2_d = param("dist2", (P, NC, L))
    wq_d = param("wq", (NL, P, ND, ND, P), bf16)   # [l, p, hp, k, m]
    wk_d = param("wk", (NL, P, ND, ND, P), bf16)
    wv_d = param("wv", (NL, P, ND, D), bf16)       # [l, p, k, n]
    bqk_d = param("bqk", (P, NL, 2, ND))           # [p, l, (q|k), hp]
    wsig_d = param("wsig", (NL, P, ND, HH), f32r)
    bsigb_d = param("bsigb", (P, NL, HH))
    wo_d = param("wo", (NL, P, ND, D), bf16)       # [l, p, hp, n]
    conv1_d = param("conv1", (NL, NK, P, ND, P), bf16)  # [l, kd, p, k, m]
    c1b_d = param("c1b", (P, NL, NK))
    conv2_d = param("conv2", (NL, NK, P, D), bf16)
    bc512_d = param("bc512", (NL, P, 7, D))        # bv,bo,c2b,ln1g,ln1b,ln2g,ln2b
    lnf_d = param("lnf", (P, 2, D))
    projw_d = param("projw", (P, ND, CIN), bf16)
    projb_d = param("projb", (P, CIN))

    series_o = nc.declare_dram_parameter("series_o", [NL, HH, L, L], f32, isOutput=True)
    prior_o = nc.declare_dram_parameter("prior_o", [NL, HH, L, L], f32, isOutput=True)
    out_o = nc.declare_dram_parameter("out_o", [L, CIN], f32, isOutput=True)

    AluOp = mybir.AluOpType
    Act = mybir.ActivationFunctionType

    # register float constants used as activation biases
    for cval in (LN3 * 1e-5, 1e-5):
        t = nc.alloc_sbuf_tensor(f"const-f32-{cval}", [128, 1], f32)
        nc.gpsimd.memset(t.ap(), cval)
        nc.const_aps.aps[(f32, cval)] = t.ap()
    nc.all_engine_barrier()

    with TileContext(nc) as tc:
        with (
            tc.tile_pool(name="const", bufs=1) as constp,
            tc.tile_pool(name="acts", bufs=1) as actp,
            tc.tile_pool(name="wstream", bufs=2) as wsp,
            tc.tile_pool(name="small", bufs=2) as smp,
            tc.tile_pool(name="heads", bufs=4) as hdp,
            tc.tile_pool(name="dram", bufs=4, space="DRAM") as drp,
        ):
            # ---- constants ----
            ident = constp.tile([P, P], f32, name="ident")
            make_identity(nc, ident)
            ident_bf = constp.tile([P, P], bf16, name="ident_bf")
            make_identity(nc, ident_bf)
            dist2_sb = constp.tile([P, NC, L], f32, name="dist2_sb")
            nc.sync.dma_start(dist2_sb[:], dist2_d[:])
            bqk_sb = constp.tile([P, NL, 2, ND], f32, name="bqk_sb")
            nc.sync.dma_start(bqk_sb[:], bqk_d[:])
            c1b_sb = constp.tile([P, NL, NK], f32, name="c1b_sb")
            nc.sync.dma_start(c1b_sb[:], c1b_d[:])
            bsig_sb = constp.tile([P, NL, HH], f32, name="bsig_sb")
            nc.sync.dma_start(bsig_sb[:], bsigb_d[:])

            def new_enc():
                return actp.tile([P, NC, D], f32, tag="enc", name="enc")

            def new_encT():
                return actp.tile([P, ND, L], bf16, tag="encT", name="encT")

            def transpose_512(dst, src, tpool, tbufs=2, dst32=None):
                # dst[p, k, l] (bf16) = transpose of src (f32, [p, c, d])
                for k in range(ND):
                    tp = tpool.tile([P, L], f32, tag="tp", bufs=tbufs, name="tp")
                    for c in range(NC):
                        nc.tensor.transpose(
                            tp[:, c * P:(c + 1) * P],
                            src[:, c, k * P:(k + 1) * P],
                            ident[:],
                        )
                    nc.vector.tensor_copy(dst[:, k, :], tp[:])
                    if dst32 is not None:
                        nc.scalar.activation(dst32[:, k, :], tp[:], Act.Copy)

            # ---- embedding ----
            with tc.tile_pool(name="embed", bufs=1) as ep:
                xcatT_sb = ep.tile([3 * CIN, L], f32r, name="xcatT_sb")
                nc.sync.dma_start(xcatT_sb[:], xcatT_d[:])
                tokw_sb = ep.tile([3 * CIN, D], f32r, name="tokw_sb")
                nc.sync.dma_start(tokw_sb[:], tokw_d[:])
                pos_sb = ep.tile([P, NC, D], f32, name="pos_sb")
                nc.sync.dma_start(pos_sb[:], pos_d[:])

                enc = new_enc()
                with tc.tile_pool(name="pemb", bufs=1, space="PSUM") as pe_p:
                    for c in range(NC):
                        emb = pe_p.tile([P, D], f32, tag="mm1", bufs=2, name="emb")
                        nc.tensor.matmul(
                            emb[:], xcatT_sb[:, c * P:(c + 1) * P], tokw_sb[:],
                            start=True, stop=True,
                        )
                        nc.vector.tensor_tensor(
                            enc[:, c, :], emb[:], pos_sb[:, c, :], AluOp.add
                        )
            def new_encT32():
                return actp.tile([P, ND, L], f32r, tag="encT32", name="encT32")

            encT = new_encT()
            encT32 = new_encT32()
            with tc.tile_pool(name="ptr0", bufs=1, space="PSUM") as tp_p:
                transpose_512(encT, enc, tp_p, dst32=encT32)

            # ---- layers ----
            for l in range(NL):
                # --- stream this layer's weights (contiguous pre-tiled) ---
                wv_t = wsp.tile([P, ND, D], bf16, tag="wv", name="wv_t")
                nc.sync.dma_start(wv_t[:], wv_d[l])
                wq_t = wsp.tile([P, ND, ND, P], bf16, tag="wq", name="wq_t")
                nc.sync.dma_start(wq_t[:], wq_d[l])
                wk_t = wsp.tile([P, ND, ND, P], bf16, tag="wk", name="wk_t")
                nc.sync.dma_start(wk_t[:], wk_d[l])
                wo_t = wsp.tile([P, ND, D], bf16, tag="wo", name="wo_t")
                nc.sync.dma_start(wo_t[:], wo_d[l])
                wsig_t = wsp.tile([P, ND, HH], f32r, tag="wsig", name="wsig_t")
                nc.sync.dma_start(wsig_t[:], wsig_d[l])
                bc = wsp.tile([P, 7, D], f32, tag="bc", bufs=1, name="bc")
                nc.sync.dma_start(bc[:], bc512_d[l])
                bo_b, c2b_b = bc[:, 1, :], bc[:, 2, :]
                ln1g_b, ln1b_b = bc[:, 3, :], bc[:, 4, :]
                ln2g_b, ln2b_b = bc[:, 5, :], bc[:, 6, :]

                pAB = tc.alloc_tile_pool(name="pAB", bufs=1, space="PSUM")

                # --- v (natural: [p, s-chunk, head, e|1], bf16, col 64 = 1.0
                # so the attnT matmul also produces the softmax sums) ---
                v_sb = actp.tile([P, NC, H, E + 1], bf16, tag="vsb", name="v_sb")
                nc.gpsimd.memset(v_sb[:, :, :, E:E + 1], 1.0)
                for sc in range(NC):
                    vp = pAB.tile([P, D], f32, tag="qkt", bufs=1, name="vp")
                    for k in range(ND):
                        nc.tensor.matmul(
                            vp[:], encT[:, k, sc * P:(sc + 1) * P], wv_t[:, k, :],
                            start=(k == 0), stop=(k == ND - 1),
                        )
                    nc.scalar.activation(
                        v_sb[:, sc, :, 0:E],
                        vp[:].rearrange("p (h e) -> p h e", h=H),
                        Act.Copy,
                    )

                # --- sig -> sigma -> a, lnc (batched over chunks) ---
                siga = smp.tile([P, NC, HH], f32, tag="siga", name="siga")
                sigc = smp.tile([P, NC, HH], f32, tag="sigc", name="sigc")
                sgall = smp.tile([P, NC, HH], f32, tag="sg", bufs=2, name="sgall")
                for c in range(NC):
                    sp = pAB.tile([P, HH], f32, tag="qkt", bufs=1, name="sp")
                    for k in range(ND):
                        nc.tensor.matmul(
                            sp[:], encT32[:, k, c * P:(c + 1) * P], wsig_t[:, k, :],
                            start=(k == 0), stop=(k == ND - 1),
                        )
                    nc.vector.tensor_tensor(
                        sgall[:, c, :], sp[:], bsig_sb[:, l, :], AluOp.add
                    )
                # sigma = 3**(sigmoid(5*sig)+1e-5) - 1
                nc.scalar.activation(sgall[:], sgall[:], Act.Sigmoid, scale=5.0)
                nc.scalar.activation(sgall[:], sgall[:], Act.Exp, scale=LN3, bias=LN3 * 1e-5)
                nc.vector.tensor_scalar(sgall[:], sgall[:], 1.0, None, AluOp.subtract)
                s2all = smp.tile([P, NC, HH], f32, tag="s2", bufs=2, name="s2all")
                nc.vector.tensor_tensor(s2all[:], sgall[:], sgall[:], AluOp.mult)
                nc.vector.reciprocal(s2all[:], s2all[:])
                nc.vector.tensor_scalar(siga[:], s2all[:], -0.5, None, AluOp.mult)
                # lnc = -ln(sigma) + ln(1/sqrt(2pi))
                nc.scalar.activation(sgall[:], sgall[:], Act.Ln)
                nc.vector.tensor_scalar(
                    sigc[:], sgall[:], -1.0, LN_INV_SQRT_2PI, AluOp.mult, AluOp.add
                )

                # --- prior (output heads) ---
                def emit_prior(l=l, siga=siga, sigc=sigc):
                    for h in range(HH):
                        pr = hdp.tile([P, NC, L], f32, tag="prior", bufs=2, name="pr")
                        for c in range(NC):
                            nc.scalar.activation(
                                pr[:, c, :], dist2_sb[:, c, :], Act.Exp,
                                scale=siga[:, c, h:h + 1],
                                bias=sigc[:, c, h:h + 1],
                            )
                        nc.gpsimd.dma_start(
                            prior_o[l, h].rearrange("(c p) s -> p c s", p=P), pr[:]
                        )

                # --- attention: q/k projections for all pairs up front ---
                attnT2 = []
                qsA = []
                ksA = []
                for hp in range(ND):
                    for j, wt, lst in ((0, wq_t, qsA), (1, wk_t, ksA)):
                        qp = pAB.tile([P, L], f32, tag="qkt", bufs=1, name="qp")
                        for k in range(ND):
                            nc.tensor.matmul(
                                qp[:], wt[:, hp, k, :], encT[:, k, :],
                                start=(k == 0), stop=(k == ND - 1),
                            )
                        qs = smp.tile([P, L], bf16, tag=f"qk{j}", bufs=4, name="qs")
                        nc.vector.tensor_scalar_add(
                            qs[:], qp[:], bqk_sb[:, l, j, hp:hp + 1]
                        )
                        lst.append(qs)

                def pe_warm(n=1):
                    w = pAB.tile([P, 256], f32, tag="warm", bufs=1, name="warm")
                    for _ in range(n):
                        nc.tensor.matmul(
                            w[:], ident[:], dist2_sb[:, 0, 0:256],
                            start=True, stop=True, skip_group_check=True,
                        )

                for hp in range(ND):
                    qT2, kT2 = qsA[hp], ksA[hp]
                    a2 = hdp.tile([P, L], bf16, tag="attnT2", name="a2")
                    for h2 in range(2):
                        hh = hp * 2 + h2
                        pe_warm(2)
                        base = h2 * 64
                        qT = qT2[base:base + 64, :]
                        kT = kT2[base:base + 64, :]
                        # scores^T natively: (s-part, l-free)
                        expTs = []
                        for sc in range(NC):
                            stp = pAB.tile([P, L], f32, tag="scT", bufs=2, name="stp")
                            nc.tensor.matmul(
                                stp[:], kT[:, sc * P:(sc + 1) * P], qT,
                                start=True, stop=True,
                            )
                            exT = hdp.tile([P, L], bf16, tag="expT", bufs=6, name="exT")
                            nc.scalar.activation(exT[:], stp[:], Act.Exp, scale=0.125)
                            expTs.append(exT)
                        # attnT accumulation straight from expT; row 64 = sums
                        ap = pAB.tile([E + 1, L], f32, tag="attnT", bufs=2, name="ap")
                        for sc in range(NC):
                            nc.tensor.matmul(
                                ap[:],
                                v_sb[:, sc, hh, :],
                                expTs[sc][:],
                                start=(sc == 0), stop=(sc == NC - 1),
                            )
                        pe_warm(2)
                        rrow = smp.tile([1, L], f32, tag="rrow", bufs=4, name="rrow")
                        nc.vector.reciprocal(rrow[:], ap[E:E + 1, :])
                        rdh = drp.tile([L], f32, tag="rd", name="rdh")
                        nc.gpsimd.dma_start(rdh[:].unsqueeze(0), rrow[:])
                        rbh = smp.tile([P, L], f32, tag="recipb", bufs=3, name="rbh")
                        nc.gpsimd.dma_start(
                            rbh[:], rdh[:].unsqueeze(0).partition_broadcast(P)
                        )
                        nc.vector.tensor_tensor(
                            a2[base:base + 64, :], ap[0:E, :], rbh[base:base + 64, :],
                            AluOp.mult,
                        )
                        if hp < 2:
                            # normalize in transposed space, transpose back, store
                            serT = hdp.tile([P, NC, L], bf16, tag="serT", bufs=1, name="serT")
                            for sc in range(NC):
                                nc.vector.tensor_tensor(
                                    serT[:, sc, :], expTs[sc][:], rbh[:], AluOp.mult
                                )
                            serp = pAB.tile([P, NC, L], bf16, tag="tr", bufs=1, name="serp")
                            for c in range(NC):
                                for sc in range(NC):
                                    nc.tensor.transpose(
                                        serp[:, c, sc * P:(sc + 1) * P],
                                        serT[:, sc, c * P:(c + 1) * P],
                                        ident_bf[:],
                                    )
                            ser = hdp.tile([P, NC, L], f32, tag="ser", bufs=1, name="ser")
                            nc.scalar.activation(ser[:], serp[:], Act.Copy)
                            nc.gpsimd.dma_start(
                                series_o[l, hh].rearrange("(c p) s -> p c s", p=P),
                                ser[:],
                            )
                    attnT2.append(a2)
                pAB.release()

                # --- Wo + residual + LN1 -> x1 ---
                pCD = tc.alloc_tile_pool(name="pCD", bufs=1, space="PSUM")
                x1 = actp.tile([P, NC, D], f32, tag="x1", name="x1")
                zres = actp.tile([P, NC, D], f32, tag="zres", name="zres")
                for c in range(NC):
                    wp = pCD.tile([P, D], f32, tag="mm1", bufs=1, name="wp")
                    for hp in range(ND):
                        nc.tensor.matmul(
                            wp[:], attnT2[hp][:, c * P:(c + 1) * P], wo_t[:, hp, :],
                            start=(hp == 0), stop=(hp == ND - 1),
                        )
                    z = zres[:, c, :]
                    nc.vector.tensor_tensor(z, wp[:], enc[:, c, :], AluOp.add)
                    nc.vector.tensor_tensor(z, z, bo_b, AluOp.add)
                _batch_layernorm(nc, smp, x1, zres, ln1g_b, ln1b_b, Act, AluOp)

                x1T = actp.tile([P, ND, L], bf16, tag="x1T", name="x1T")
                transpose_512(x1T, x1, pCD, tbufs=1)

                # --- FFN ---
                ency = pCD.tile([P, NC, D], f32, tag="ency", name="ency")
                for kd in range(NK):
                    if kd % 4 == 0:
                        c1q = wsp.tile([P, 4, ND, P], bf16, tag="c1", bufs=2, name="c1q")
                        nc.sync.dma_start(
                            c1q[:],
                            conv1_d[l, kd:kd + 4].rearrange("q p k m -> p q k m"),
                        )
                        c2q = wsp.tile([P, 4, D], bf16, tag="c2", bufs=2, name="c2q")
                        nc.sync.dma_start(
                            c2q[:],
                            conv2_d[l, kd:kd + 4].rearrange("q p n -> p q n"),
                        )
                    yp = pCD.tile([P, L], f32, tag="yT", bufs=2, name="yp")
                    for k in range(ND):
                        nc.tensor.matmul(
                            yp[:], c1q[:, kd % 4, k, :], x1T[:, k, :],
                            start=(k == 0), stop=(k == ND - 1),
                        )
                    gt = wsp.tile([P, L], bf16, tag="gt", bufs=4, name="gt")
                    nc.scalar.activation(
                        gt[:], yp[:], Act.Gelu, bias=c1b_sb[:, l, kd:kd + 1]
                    )
                    for c in range(NC):
                        nc.tensor.matmul(
                            ency[:, c, :], gt[:, c * P:(c + 1) * P], c2q[:, kd % 4, :],
                            start=(kd == 0), stop=(kd == NK - 1),
                        )

                emit_prior()

                # --- residual + LN2 -> enc (next layer) ---
                enc = new_enc()
                for c in range(NC):
                    z = zres[:, c, :]
                    nc.vector.tensor_tensor(z, ency[:, c, :], x1[:, c, :], AluOp.add)
                    nc.vector.tensor_tensor(z, z, c2b_b, AluOp.add)
                _batch_layernorm(nc, smp, enc, zres, ln2g_b, ln2b_b, Act, AluOp)
                encT = new_encT()
                encT32 = new_encT32()
                transpose_512(encT, enc, pCD, tbufs=1, dst32=encT32)
                pCD.release()

            # ---- final LN + projection ----
            lnf_sb = constp.tile([P, 2, D], f32, name="lnf_sb")
            nc.sync.dma_start(lnf_sb[:], lnf_d[:])
            projw_t = constp.tile([P, ND, CIN], bf16, name="projw_t")
            nc.sync.dma_start(projw_t[:], projw_d[:])
            projb_sb = constp.tile([P, CIN], f32, name="projb_sb")
            nc.sync.dma_start(projb_sb[:], projb_d[:])

            encf = actp.tile([P, NC, D], f32, tag="x1", name="encf")
            _batch_layernorm(nc, smp, encf, enc, lnf_sb[:, 0, :], lnf_sb[:, 1, :], Act, AluOp)
            pF = tc.alloc_tile_pool(name="pF", bufs=1, space="PSUM")
            encfT = actp.tile([P, ND, L], bf16, tag="x1T", name="encfT")
            transpose_512(encfT, encf, pF)
            for c in range(NC):
                op = pF.tile([P, CIN], f32, tag="outp", bufs=2, name="op")
                for k in range(ND):
                    nc.tensor.matmul(
                        op[:], encfT[:, k, c * P:(c + 1) * P], projw_t[:, k, :],
                        start=(k == 0), stop=(k == ND - 1),
                    )
                ot = smp.tile([P, CIN], f32, tag="ot", bufs=2, name="ot")
                nc.vector.tensor_tensor(ot[:], op[:], projb_sb[:], AluOp.add)
                nc.gpsimd.dma_start(out_o[c * P:(c + 1) * P, :], ot[:])
            pF.release()

    _fix_multi_waits(nc)
    return nc


def _batch_layernorm(nc, smp, out_t, z_t, g_b, b_b, Act, AluOp):
    """LayerNorm over free dim for all 4 chunks; single Sqrt/recip batch.
    out_t/z_t: [P, NC, D] tiles."""
    import concourse.mybir as mybir

    f32 = mybir.dt.float32
    mv = smp.tile([P, NC, 2], f32, tag="mv", bufs=2, name="mv")
    for c in range(NC):
        st6 = smp.tile([P, 6], f32, tag="st6", bufs=2, name="st6")
        nc.vector.bn_stats(st6[:], z_t[:, c, :])
        nc.vector.bn_aggr(mv[:, c, :], st6[:])
    rstd = smp.tile([P, NC], f32, tag="rstd", bufs=2, name="rstd")
    nc.scalar.activation(rstd[:], mv[:, :, 1], Act.Sqrt, bias=1e-5)
    nc.vector.reciprocal(rstd[:], rstd[:])
    for c in range(NC):
        o = out_t[:, c, :]
        nc.vector.tensor_scalar(
            o, z_t[:, c, :], mv[:, c, 0:1], rstd[:, c:c + 1],
            AluOp.subtract, AluOp.mult,
        )
        nc.vector.tensor_tensor(o, o, g_b, AluOp.mult)
        nc.vector.tensor_tensor(o, o, b_b, AluOp.add)


# ---------------------------------------------------------------------------
# Host-side staging
# ---------------------------------------------------------------------------

def _pos_embedding():
    pos = np.arange(L, dtype=np.float32)[:, None]
    div = np.exp(np.arange(0, D, 2, dtype=np.float32) * -(math.log(10000.0) / D))
    pe = np.zeros((L, D), dtype=np.float32)
    pe[:, 0::2] = np.sin(pos * div)
    pe[:, 1::2] = np.cos(pos * div)
    return pe


def _bcast(v):
    return np.broadcast_to(np.asarray(v, np.float32)[None, :], (P, v.shape[-1])).copy()


def _stage_shared(p):
    """Staging that is identical for every core (computed once)."""
    import ml_dtypes

    bf16 = ml_dtypes.bfloat16

    c1b = np.asarray(p['conv1_b'], np.float32)  # (NL, DFF)
    c1bt = np.zeros((P, NL, NK), np.float32)
    for l in range(NL):
        c1bt[:, l, :] = c1b[l].reshape(NK, P).T

    # conv1: (NL, D, DFF) -> [l, kd, p, k, m]
    conv1 = np.asarray(p['conv1_w'], np.float32).reshape(NL, ND, P, NK, P)
    conv1 = np.ascontiguousarray(conv1.transpose(0, 3, 2, 1, 4)).astype(bf16)
    # conv2: (NL, DFF, D) -> [l, kd, p, n]
    conv2 = np.ascontiguousarray(
        np.asarray(p['conv2_w'], np.float32).reshape(NL, NK, P, D)
    ).astype(bf16)

    lnf = np.zeros((P, 2, D), np.float32)
    lnf[:, 0, :] = _bcast(np.asarray(p['lnf_g'], np.float32))
    lnf[:, 1, :] = _bcast(np.asarray(p['lnf_b'], np.float32))

    projw = np.asarray(p['proj_w'], np.float32).reshape(ND, P, CIN)
    projw = np.ascontiguousarray(projw.transpose(1, 0, 2)).astype(bf16)

    idx = np.arange(L, dtype=np.float32)
    dist2 = ((idx[:, None] - idx[None, :]) ** 2).reshape(NC, P, L)
    dist2 = np.ascontiguousarray(dist2.transpose(1, 0, 2))

    pos = _pos_embedding().reshape(NC, P, D)
    pos = np.ascontiguousarray(pos.transpose(1, 0, 2))

    return {
        'tokw': np.ascontiguousarray(
            np.asarray(p['tok_w'], np.float32).reshape(3 * CIN, D)
        ),
        'pos': pos,
        'dist2': dist2,
        'conv1': conv1,
        'c1b': c1bt,
        'conv2': conv2,
        'lnf': lnf,
        'projw': projw,
        'projb': _bcast(np.asarray(p['proj_b'], np.float32)),
    }


def _stage_core(x, p, b, half, shared):
    import ml_dtypes

    bf16 = ml_dtypes.bfloat16

    xb = np.asarray(x[b], dtype=np.float32)
    xcat = np.concatenate(
        [np.roll(xb, 1, axis=0), xb, np.roll(xb, -1, axis=0)], axis=1
    )

    def permc(w):
        w = np.asarray(w, np.float32)
        wh = w.reshape(w.shape[:-1] + (H, w.shape[-1] // H))
        if half:
            wh = np.concatenate([wh[..., HH:, :], wh[..., :HH, :]], axis=-2)
        return np.ascontiguousarray(wh.reshape(w.shape))

    # wq/wk: (NL, D, D) -> [l, p, hp, k, m]   (k = contraction d-tile)
    wq = permc(p['Wq']).reshape(NL, ND, P, ND, P)
    wq = np.ascontiguousarray(wq.transpose(0, 2, 3, 1, 4)).astype(bf16)
    wk = permc(p['Wk']).reshape(NL, ND, P, ND, P)
    wk = np.ascontiguousarray(wk.transpose(0, 2, 3, 1, 4)).astype(bf16)
    wv = permc(p['Wv']).reshape(NL, ND, P, D)
    wv = np.ascontiguousarray(wv.transpose(0, 2, 1, 3)).astype(bf16)

    wsig = np.asarray(p['Wsig'], np.float32)
    bsig = np.asarray(p['bsig'], np.float32)
    if half:
        wsig, bsig = wsig[:, :, HH:], bsig[:, HH:]
    else:
        wsig, bsig = wsig[:, :, :HH], bsig[:, :HH]
    wsig = wsig.reshape(NL, ND, P, HH)
    wsig = np.ascontiguousarray(wsig.transpose(0, 2, 1, 3))

    # wo rows permuted: (NL, D, D) -> [l, p, hp, n]
    wo = np.asarray(p['Wo'], np.float32)
    woh = wo.reshape(NL, H, E, D)
    if half:
        woh = np.concatenate([woh[:, HH:], woh[:, :HH]], axis=1)
    wo = woh.reshape(NL, ND, P, D)
    wo = np.ascontiguousarray(wo.transpose(0, 2, 1, 3)).astype(bf16)

    bq = permc(p['bq'])
    bk = permc(p['bk'])
    bqk = np.zeros((P, NL, 2, ND), np.float32)
    for l in range(NL):
        for hp in range(ND):
            bqk[:, l, 0, hp] = bq[l, hp * P:(hp + 1) * P]
            bqk[:, l, 1, hp] = bk[l, hp * P:(hp + 1) * P]

    bv0 = np.asarray(p['bv'], np.float32)
    wo0 = np.asarray(p['Wo'], np.float32)
    bo_eff = np.asarray(p['bo'], np.float32) + np.einsum('ld,ldn->ln', bv0, wo0)
    bc512 = np.zeros((NL, P, 7, D), np.float32)
    for l in range(NL):
        for i, vec in enumerate([
            np.zeros(D, np.float32), bo_eff[l],
            np.asarray(p['conv2_b'], np.float32)[l],
            np.asarray(p['ln1_g'], np.float32)[l], np.asarray(p['ln1_b'], np.float32)[l],
            np.asarray(p['ln2_g'], np.float32)[l], np.asarray(p['ln2_b'], np.float32)[l],
        ]):
            bc512[l, :, i, :] = np.broadcast_to(vec[None, :], (P, D))

    bsigb = np.zeros((P, NL, HH), np.float32)
    for l in range(NL):
        bsigb[:, l, :] = np.broadcast_to(bsig[l][None, :], (P, HH))

    return {
        'xcatT': np.ascontiguousarray(xcat.T),
        'tokw': shared['tokw'],
        'pos': shared['pos'],
        'dist2': shared['dist2'],
        'wq': wq, 'wk': wk, 'wv': wv,
        'bqk': bqk,
        'wsig': wsig, 'bsigb': bsigb,
        'wo': wo,
        'conv1': shared['conv1'],
        'c1b': shared['c1b'],
        'conv2': shared['conv2'],
        'bc512': bc512,
        'lnf': shared['lnf'],
        'projw': shared['projw'],
        'projb': shared['projb'],
    }


def kernel(x=None, params=None, **kw):
    from concourse.bass_utils import run_bass_kernel_spmd

    _patch_drain_wait_split()
    if 'nc' not in _CACHE:
        _CACHE['nc'] = _build_nc()
    nc = _CACHE['nc']

    x = np.asarray(x, np.float32)
    p = {k: np.asarray(v) for k, v in params.items()}

    shared = _stage_shared(p)
    in_maps = [_stage_core(x, p, c // 2, c % 2, shared) for c in range(8)]

    res = run_bass_kernel_spmd(nc, in_maps, list(range(8)))
    results = res.results

    out = np.zeros((B, L, CIN), np.float32)
    series = np.zeros((NL, B, H, L, L), np.float32)
    prior = np.zeros((NL, B, H, L, L), np.float32)
    for core in range(8):
        b, half = core // 2, core % 2
        rr = results[core]
        if half == 0:
            out[b] = rr["out_o"]
        series[:, b, half * HH:(half + 1) * HH] = rr["series_o"]
        prior[:, b, half * HH:(half + 1) * HH] = rr["prior_o"]
    return out, series, prior


# revision 29
# speedup vs baseline: 1.2683x; 1.2683x over previous
"""AnomalyTransformer forward on 8 TRN2 NeuronCores.

Sharding: collective-free. Core c computes batch element b=c//2 end-to-end
(pairs duplicate the per-token pipeline), but each core materializes/writes
only 4 of the 8 attention heads' series/prior tensors. Odd cores see the
head-blocks of Wq/Wk/Wv/Wsig (cols) and Wo (rows) pre-permuted host-side so
the SPMD graph always outputs "heads 0-3" of its own view.

Precision: residual stream, LayerNorms, softmax sums, series/prior outputs in
fp32; Q/K/V projections, attention and FFN matmuls in bf16 (weights stored
bf16 in HBM, halving weight reads); Wo and the final projection in float32r
(tf32-like full-rate fp32). Softmax skips max-subtraction (|scores| <~ 1 by
construction). The softmax recip is folded per-partition after exp (series
output) and after the unnormalized attn^T matmul, so series^T is obtained by
SBUF->SBUF DMA-transposes of the bf16 exp tiles instead of a second matmul
and exp pass.

All weight/constant arrays are pre-tiled host-side so every DMA is one
contiguous 128-partition block.
"""
import sys

sys.path.insert(0, '/opt/trn_rl_repo')

import math
import numpy as np

B, L, CIN, D, H, NL, DFF = 4, 512, 38, 512, 8, 3, 2048
E = D // H           # 64
HH = H // 2          # 4 output heads per core
NC = 4               # 128-token chunks per L
ND = 4               # 128-row chunks per D
NK = DFF // 128      # 16 dff chunks
P = 128
LN3 = math.log(3.0)
INV_SQRT_2PI = 1.0 / math.sqrt(2.0 * math.pi)
LN_INV_SQRT_2PI = -0.5 * math.log(2.0 * math.pi)

_CACHE = {}


# ---------------------------------------------------------------------------
# walrus on this image rejects instructions with more than one sync wait.
# Split excess waits onto nofuse nops on the same engine.
# ---------------------------------------------------------------------------

def _patch_drain_wait_split():
    from concourse import tile as tile_mod
    import bass_rust

    if getattr(tile_mod.TileContext._drain_and_barrier, '_waitfix', False):
        return

    def patched(self, tick_clock, wait_clock):
        from concourse.vector_clock import ScopedClock

        nc = self.nc
        drain_inst = nc.sync.drain()
        wait_clock.add_sem_waits(
            drain_inst.ins, ScopedClock({None: tick_clock.global_clock})
        )
        si = drain_inst.ins.sync_info
        waits = list(si.on_wait)
        if len(waits) > 1:
            si.on_wait = waits[:1]
            drain_inst.ins.sync_info = si
            for w in waits[1:]:
                nop = nc.sync.nop(nofuse=True, hint="drain_wait_split")
                nop.ins.sync_info = bass_rust.SyncInfo(on_wait=[w], on_update=[])

        nc.all_engine_barrier()
        assert self.sems is not None
        popped = nc._tile_sem_poison_stack.pop()
        assert popped is self._sem_poison
        nc.clear_and_free_semaphores(list(self.sems.allocated().values()))
        nc.all_engine_barrier()

    patched._waitfix = True
    tile_mod.TileContext._drain_and_barrier = patched


def _fix_multi_waits(nc):
    """Post-pass: any instruction carrying >1 sem waits gets the extra waits
    hoisted onto nofuse nops inserted just before it (same engine)."""
    from concourse import mybir

    n = 0
    for f in nc.m.functions:
        for bb in f.blocks:
            insts = list(bb.instructions)
            out = []
            changed = False
            for ins in insts:
                si = ins.sync_info
                if si is not None and len(si.on_wait) > 1:
                    waits = list(si.on_wait)
                    for w in waits[:-1]:
                        n += 1
                        nop = mybir.InstNoOp(
                            name=f"waitfix-{n}",
                            engine=ins.engine,
                            bass_nofuse=True,
                            sync_info=mybir.SyncInfo(on_wait=[w], on_update=[]),
                        )
                        out.append(nop)
                    si.on_wait = waits[-1:]
                    ins.sync_info = si
                    changed = True
                out.append(ins)
            if changed:
                bb.instructions = out
    return n


# ---------------------------------------------------------------------------
# Device graph
# ---------------------------------------------------------------------------

def _build_nc():
    import concourse.bass as bass
    import concourse.mybir as mybir
    from concourse.tile import TileContext
    from concourse.masks import make_identity

    f32 = mybir.dt.float32
    f32r = mybir.dt.float32r
    bf16 = mybir.dt.bfloat16
    nc = bass.Bass()

    def param(name, shape, dt=None):
        return nc.declare_dram_parameter(
            name, list(shape), dt or f32, isOutput=False
        )

    # pre-tiled host layouts (see _stage_core)
    xcatT_d = param("xcatT", (3 * CIN, L), f32r)
    tokw_d = param("tokw", (3 * CIN, D), f32r)
    pos_d = param("pos", (P, NC, D))
    dist2_d = param("dist2", (P, NC, L))
    wq_d = param("wq", (NL, P, ND, ND, P), bf16)   # [l, p, hp, k, m]
    wk_d = param("wk", (NL, P, ND, ND, P), bf16)
    wv_d = param("wv", (NL, P, ND, D), bf16)       # [l, p, k, n]
    bqk_d = param("bqk", (P, NL, 2, ND))           # [p, l, (q|k), hp]
    wsig_d = param("wsig", (NL, P, ND, HH), f32r)
    bsigb_d = param("bsigb", (P, NL, HH))
    wo_d = param("wo", (NL, P, ND, D), bf16)       # [l, p, hp, n]
    conv1_d = param("conv1", (NL, NK, P, ND, P), bf16)  # [l, kd, p, k, m]
    c1b_d = param("c1b", (P, NL, NK))
    conv2_d = param("conv2", (NL, NK, P, D), bf16)
    bc512_d = param("bc512", (NL, P, 7, D))        # bv,bo,c2b,ln1g,ln1b,ln2g,ln2b
    lnf_d = param("lnf", (P, 2, D))
    projw_d = param("projw", (P, ND, CIN), bf16)
    projb_d = param("projb", (P, CIN))

    series_o = nc.declare_dram_parameter("series_o", [NL, HH, L, L], f32, isOutput=True)
    prior_o = nc.declare_dram_parameter("prior_o", [NL, HH, L, L], f32, isOutput=True)
    out_o = nc.declare_dram_parameter("out_o", [L, CIN], f32, isOutput=True)

    AluOp = mybir.AluOpType
    Act = mybir.ActivationFunctionType

    # register float constants used as activation biases
    for cval in (LN3 * 1e-5, 1e-5):
        t = nc.alloc_sbuf_tensor(f"const-f32-{cval}", [128, 1], f32)
        nc.gpsimd.memset(t.ap(), cval)
        nc.const_aps.aps[(f32, cval)] = t.ap()
    nc.all_engine_barrier()

    with TileContext(nc) as tc:
        with (
            tc.tile_pool(name="const", bufs=1) as constp,
            tc.tile_pool(name="acts", bufs=1) as actp,
            tc.tile_pool(name="wstream", bufs=2) as wsp,
            tc.tile_pool(name="small", bufs=2) as smp,
            tc.tile_pool(name="heads", bufs=4) as hdp,
            tc.tile_pool(name="dram", bufs=4, space="DRAM") as drp,
        ):
            # ---- constants ----
            ident = constp.tile([P, P], f32, name="ident")
            make_identity(nc, ident)
            ident_bf = constp.tile([P, P], bf16, name="ident_bf")
            make_identity(nc, ident_bf)
            onesc_bf = constp.tile([1, P], bf16, name="onesc_bf")
            nc.gpsimd.memset(onesc_bf[:], 1.0)
            dist2_sb = constp.tile([P, NC, L], f32, name="dist2_sb")
            nc.sync.dma_start(dist2_sb[:], dist2_d[:])
            bqk_sb = constp.tile([P, NL, 2, ND], f32, name="bqk_sb")
            nc.sync.dma_start(bqk_sb[:], bqk_d[:])
            c1b_sb = constp.tile([P, NL, NK], f32, name="c1b_sb")
            nc.sync.dma_start(c1b_sb[:], c1b_d[:])
            bsig_sb = constp.tile([P, NL, HH], f32, name="bsig_sb")
            nc.sync.dma_start(bsig_sb[:], bsigb_d[:])

            def new_enc():
                return actp.tile([P, NC, D], f32, tag="enc", name="enc")

            def new_encT():
                return actp.tile([P, ND, L], bf16, tag="encT", name="encT")

            def transpose_512(dst, src, tpool, tbufs=2, dst32=None):
                # dst[p, k, l] (bf16) = transpose of src (f32, [p, c, d])
                for k in range(ND):
                    tp = tpool.tile([P, L], f32, tag="tp", bufs=tbufs, name="tp")
                    for c in range(NC):
                        nc.tensor.transpose(
                            tp[:, c * P:(c + 1) * P],
                            src[:, c, k * P:(k + 1) * P],
                            ident[:],
                        )
                    nc.vector.tensor_copy(dst[:, k, :], tp[:])
                    if dst32 is not None:
                        nc.scalar.activation(dst32[:, k, :], tp[:], Act.Copy)

            # ---- embedding ----
            with tc.tile_pool(name="embed", bufs=1) as ep:
                xcatT_sb = ep.tile([3 * CIN, L], f32r, name="xcatT_sb")
                nc.sync.dma_start(xcatT_sb[:], xcatT_d[:])
                tokw_sb = ep.tile([3 * CIN, D], f32r, name="tokw_sb")
                nc.sync.dma_start(tokw_sb[:], tokw_d[:])
                pos_sb = ep.tile([P, NC, D], f32, name="pos_sb")
                nc.sync.dma_start(pos_sb[:], pos_d[:])

                enc = new_enc()
                with tc.tile_pool(name="pemb", bufs=1, space="PSUM") as pe_p:
                    for c in range(NC):
                        emb = pe_p.tile([P, D], f32, tag="mm1", bufs=2, name="emb")
                        nc.tensor.matmul(
                            emb[:], xcatT_sb[:, c * P:(c + 1) * P], tokw_sb[:],
                            start=True, stop=True,
                        )
                        nc.vector.tensor_tensor(
                            enc[:, c, :], emb[:], pos_sb[:, c, :], AluOp.add
                        )
            def new_encT32():
                return actp.tile([P, ND, L], f32r, tag="encT32", name="encT32")

            encT = new_encT()
            encT32 = new_encT32()
            with tc.tile_pool(name="ptr0", bufs=1, space="PSUM") as tp_p:
                transpose_512(encT, enc, tp_p, dst32=encT32)

            # ---- layers ----
            for l in range(NL):
                # --- stream this layer's weights (contiguous pre-tiled) ---
                wv_t = wsp.tile([P, ND, D], bf16, tag="wv", name="wv_t")
                nc.sync.dma_start(wv_t[:], wv_d[l])
                wq_t = wsp.tile([P, ND, ND, P], bf16, tag="wq", name="wq_t")
                nc.sync.dma_start(wq_t[:], wq_d[l])
                wk_t = wsp.tile([P, ND, ND, P], bf16, tag="wk", name="wk_t")
                nc.sync.dma_start(wk_t[:], wk_d[l])
                wo_t = wsp.tile([P, ND, D], bf16, tag="wo", name="wo_t")
                nc.sync.dma_start(wo_t[:], wo_d[l])
                wsig_t = wsp.tile([P, ND, HH], f32r, tag="wsig", name="wsig_t")
                nc.sync.dma_start(wsig_t[:], wsig_d[l])
                bc = wsp.tile([P, 7, D], f32, tag="bc", bufs=1, name="bc")
                nc.sync.dma_start(bc[:], bc512_d[l])
                bo_b, c2b_b = bc[:, 1, :], bc[:, 2, :]
                ln1g_b, ln1b_b = bc[:, 3, :], bc[:, 4, :]
                ln2g_b, ln2b_b = bc[:, 5, :], bc[:, 6, :]

                pAB = tc.alloc_tile_pool(name="pAB", bufs=1, space="PSUM")

                # --- v (natural: [p, s-chunk, head, e|1], bf16, col 64 = 1.0
                # so the attnT matmul also produces the softmax sums) ---
                v_sb = actp.tile([P, NC, H, E + 1], bf16, tag="vsb", name="v_sb")
                nc.gpsimd.memset(v_sb[:, :, :, E:E + 1], 1.0)
                for sc in range(NC):
                    vp = pAB.tile([P, D], f32, tag="qkt", bufs=1, name="vp")
                    for k in range(ND):
                        nc.tensor.matmul(
                            vp[:], encT[:, k, sc * P:(sc + 1) * P], wv_t[:, k, :],
                            start=(k == 0), stop=(k == ND - 1),
                        )
                    nc.scalar.activation(
                        v_sb[:, sc, :, 0:E],
                        vp[:].rearrange("p (h e) -> p h e", h=H),
                        Act.Copy,
                    )

                # --- sig -> sigma -> a, lnc (batched over chunks) ---
                siga = smp.tile([P, NC, HH], f32, tag="siga", name="siga")
                sigc = smp.tile([P, NC, HH], f32, tag="sigc", name="sigc")
                sgall = smp.tile([P, NC, HH], f32, tag="sg", bufs=2, name="sgall")
                for c in range(NC):
                    sp = pAB.tile([P, HH], f32, tag="qkt", bufs=1, name="sp")
                    for k in range(ND):
                        nc.tensor.matmul(
                            sp[:], encT32[:, k, c * P:(c + 1) * P], wsig_t[:, k, :],
                            start=(k == 0), stop=(k == ND - 1),
                        )
                    nc.vector.tensor_tensor(
                        sgall[:, c, :], sp[:], bsig_sb[:, l, :], AluOp.add
                    )
                # sigma = 3**(sigmoid(5*sig)+1e-5) - 1
                nc.scalar.activation(sgall[:], sgall[:], Act.Sigmoid, scale=5.0)
                nc.scalar.activation(sgall[:], sgall[:], Act.Exp, scale=LN3, bias=LN3 * 1e-5)
                nc.vector.tensor_scalar(sgall[:], sgall[:], 1.0, None, AluOp.subtract)
                s2all = smp.tile([P, NC, HH], f32, tag="s2", bufs=2, name="s2all")
                nc.vector.tensor_tensor(s2all[:], sgall[:], sgall[:], AluOp.mult)
                nc.vector.reciprocal(s2all[:], s2all[:])
                nc.vector.tensor_scalar(siga[:], s2all[:], -0.5, None, AluOp.mult)
                # lnc = -ln(sigma) + ln(1/sqrt(2pi))
                nc.scalar.activation(sgall[:], sgall[:], Act.Ln)
                nc.vector.tensor_scalar(
                    sigc[:], sgall[:], -1.0, LN_INV_SQRT_2PI, AluOp.mult, AluOp.add
                )

                # --- prior (output heads) ---
                def emit_prior(l=l, siga=siga, sigc=sigc):
                    for h in range(HH):
                        pr = hdp.tile([P, NC, L], f32, tag="prior", bufs=2, name="pr")
                        for c in range(NC):
                            nc.scalar.activation(
                                pr[:, c, :], dist2_sb[:, c, :], Act.Exp,
                                scale=siga[:, c, h:h + 1],
                                bias=sigc[:, c, h:h + 1],
                            )
                        nc.gpsimd.dma_start(
                            prior_o[l, h].rearrange("(c p) s -> p c s", p=P), pr[:]
                        )

                # --- attention: q/k projections for all pairs up front ---
                attnT2 = []
                qsA = []
                ksA = []
                for hp in range(ND):
                    for j, wt, lst in ((0, wq_t, qsA), (1, wk_t, ksA)):
                        qp = pAB.tile([P, L], f32, tag="qkt", bufs=1, name="qp")
                        for k in range(ND):
                            nc.tensor.matmul(
                                qp[:], wt[:, hp, k, :], encT[:, k, :],
                                start=(k == 0), stop=(k == ND - 1),
                            )
                        qs = smp.tile([P, L], bf16, tag=f"qk{j}", bufs=4, name="qs")
                        nc.vector.tensor_scalar_add(
                            qs[:], qp[:], bqk_sb[:, l, j, hp:hp + 1]
                        )
                        lst.append(qs)

                for hp in range(ND):
                    qT2, kT2 = qsA[hp], ksA[hp]
                    a2 = hdp.tile([P, L], bf16, tag="attnT2", name="a2")
                    for h2 in range(2):
                        hh = hp * 2 + h2
                        base = h2 * 64
                        qT = qT2[base:base + 64, :]
                        kT = kT2[base:base + 64, :]
                        # scores^T natively: (s-part, l-free)
                        expTs = []
                        for sc in range(NC):
                            stp = pAB.tile([P, L], f32, tag="scT", bufs=2, name="stp")
                            nc.tensor.matmul(
                                stp[:], kT[:, sc * P:(sc + 1) * P], qT,
                                start=True, stop=True,
                            )
                            exT = hdp.tile([P, L], bf16, tag="expT", bufs=6, name="exT")
                            nc.scalar.activation(exT[:], stp[:], Act.Exp, scale=0.125)
                            expTs.append(exT)
                        # attnT accumulation straight from expT; row 64 = sums
                        ap = pAB.tile([E + 1, L], f32, tag="attnT", bufs=2, name="ap")
                        for sc in range(NC):
                            nc.tensor.matmul(
                                ap[:],
                                v_sb[:, sc, hh, :],
                                expTs[sc][:],
                                start=(sc == 0), stop=(sc == NC - 1),
                            )
                        rrow = smp.tile([1, L], bf16, tag="rrow", bufs=4, name="rrow")
                        with nc.allow_low_precision(reason="softmax recip to bf16 for broadcast matmul"):
                            nc.vector.reciprocal(rrow[:], ap[E:E + 1, :])
                        rbp = pAB.tile([P, L], f32, tag="warm", bufs=1, name="rbp")
                        nc.tensor.matmul(
                            rbp[:], onesc_bf[:], rrow[:], start=True, stop=True,
                        )
                        rbh = smp.tile([P, L], f32, tag="recipb", bufs=3, name="rbh")
                        nc.vector.tensor_copy(rbh[:], rbp[:])
                        nc.vector.tensor_tensor(
                            a2[base:base + 64, :], ap[0:E, :], rbh[base:base + 64, :],
                            AluOp.mult,
                        )
                        if hp < 2:
                            # normalize in transposed space, transpose back, store
                            serT = hdp.tile([P, NC, L], bf16, tag="serT", bufs=1, name="serT")
                            for sc in range(NC):
                                nc.vector.tensor_tensor(
                                    serT[:, sc, :], expTs[sc][:], rbh[:], AluOp.mult
                                )
                            serp = pAB.tile([P, NC, L], bf16, tag="tr", bufs=1, name="serp")
                            for c in range(NC):
                                for sc in range(NC):
                                    nc.tensor.transpose(
                                        serp[:, c, sc * P:(sc + 1) * P],
                                        serT[:, sc, c * P:(c + 1) * P],
                                        ident_bf[:],
                                    )
                            ser = hdp.tile([P, NC, L], f32, tag="ser", bufs=1, name="ser")
                            nc.scalar.activation(ser[:], serp[:], Act.Copy)
                            nc.gpsimd.dma_start(
                                series_o[l, hh].rearrange("(c p) s -> p c s", p=P),
                                ser[:],
                            )
                    attnT2.append(a2)
                pAB.release()

                # --- Wo + residual + LN1 -> x1 ---
                pCD = tc.alloc_tile_pool(name="pCD", bufs=1, space="PSUM")
                x1 = actp.tile([P, NC, D], f32, tag="x1", name="x1")
                zres = actp.tile([P, NC, D], f32, tag="zres", name="zres")
                for c in range(NC):
                    wp = pCD.tile([P, D], f32, tag="mm1", bufs=1, name="wp")
                    for hp in range(ND):
                        nc.tensor.matmul(
                            wp[:], attnT2[hp][:, c * P:(c + 1) * P], wo_t[:, hp, :],
                            start=(hp == 0), stop=(hp == ND - 1),
                        )
                    z = zres[:, c, :]
                    nc.vector.tensor_tensor(z, wp[:], enc[:, c, :], AluOp.add)
                    nc.vector.tensor_tensor(z, z, bo_b, AluOp.add)
                _batch_layernorm(nc, smp, x1, zres, ln1g_b, ln1b_b, Act, AluOp)

                x1T = actp.tile([P, ND, L], bf16, tag="x1T", name="x1T")
                transpose_512(x1T, x1, pCD, tbufs=1)

                # --- FFN ---
                ency = pCD.tile([P, NC, D], f32, tag="ency", name="ency")
                for kd in range(NK):
                    if kd % 4 == 0:
                        c1q = wsp.tile([P, 4, ND, P], bf16, tag="c1", bufs=2, name="c1q")
                        nc.sync.dma_start(
                            c1q[:],
                            conv1_d[l, kd:kd + 4].rearrange("q p k m -> p q k m"),
                        )
                        c2q = wsp.tile([P, 4, D], bf16, tag="c2", bufs=2, name="c2q")
                        nc.sync.dma_start(
                            c2q[:],
                            conv2_d[l, kd:kd + 4].rearrange("q p n -> p q n"),
                        )
                    yp = pCD.tile([P, L], f32, tag="yT", bufs=2, name="yp")
                    for k in range(ND):
                        nc.tensor.matmul(
                            yp[:], c1q[:, kd % 4, k, :], x1T[:, k, :],
                            start=(k == 0), stop=(k == ND - 1),
                        )
                    gt = wsp.tile([P, L], bf16, tag="gt", bufs=4, name="gt")
                    nc.scalar.activation(
                        gt[:], yp[:], Act.Gelu, bias=c1b_sb[:, l, kd:kd + 1]
                    )
                    for c in range(NC):
                        nc.tensor.matmul(
                            ency[:, c, :], gt[:, c * P:(c + 1) * P], c2q[:, kd % 4, :],
                            start=(kd == 0), stop=(kd == NK - 1),
                        )

                emit_prior()

                # --- residual + LN2 -> enc (next layer) ---
                enc = new_enc()
                for c in range(NC):
                    z = zres[:, c, :]
                    nc.vector.tensor_tensor(z, ency[:, c, :], x1[:, c, :], AluOp.add)
                    nc.vector.tensor_tensor(z, z, c2b_b, AluOp.add)
                _batch_layernorm(nc, smp, enc, zres, ln2g_b, ln2b_b, Act, AluOp)
                encT = new_encT()
                encT32 = new_encT32()
                transpose_512(encT, enc, pCD, tbufs=1, dst32=encT32)
                pCD.release()

            # ---- final LN + projection ----
            lnf_sb = constp.tile([P, 2, D], f32, name="lnf_sb")
            nc.sync.dma_start(lnf_sb[:], lnf_d[:])
            projw_t = constp.tile([P, ND, CIN], bf16, name="projw_t")
            nc.sync.dma_start(projw_t[:], projw_d[:])
            projb_sb = constp.tile([P, CIN], f32, name="projb_sb")
            nc.sync.dma_start(projb_sb[:], projb_d[:])

            encf = actp.tile([P, NC, D], f32, tag="x1", name="encf")
            _batch_layernorm(nc, smp, encf, enc, lnf_sb[:, 0, :], lnf_sb[:, 1, :], Act, AluOp)
            pF = tc.alloc_tile_pool(name="pF", bufs=1, space="PSUM")
            encfT = actp.tile([P, ND, L], bf16, tag="x1T", name="encfT")
            transpose_512(encfT, encf, pF)
            for c in range(NC):
                op = pF.tile([P, CIN], f32, tag="outp", bufs=2, name="op")
                for k in range(ND):
                    nc.tensor.matmul(
                        op[:], encfT[:, k, c * P:(c + 1) * P], projw_t[:, k, :],
                        start=(k == 0), stop=(k == ND - 1),
                    )
                ot = smp.tile([P, CIN], f32, tag="ot", bufs=2, name="ot")
                nc.vector.tensor_tensor(ot[:], op[:], projb_sb[:], AluOp.add)
                nc.gpsimd.dma_start(out_o[c * P:(c + 1) * P, :], ot[:])
            pF.release()

    _fix_multi_waits(nc)
    return nc


def _batch_layernorm(nc, smp, out_t, z_t, g_b, b_b, Act, AluOp):
    """LayerNorm over free dim for all 4 chunks; single Sqrt/recip batch.
    out_t/z_t: [P, NC, D] tiles."""
    import concourse.mybir as mybir

    f32 = mybir.dt.float32
    mv = smp.tile([P, NC, 2], f32, tag="mv", bufs=2, name="mv")
    for c in range(NC):
        st6 = smp.tile([P, 6], f32, tag="st6", bufs=2, name="st6")
        nc.vector.bn_stats(st6[:], z_t[:, c, :])
        nc.vector.bn_aggr(mv[:, c, :], st6[:])
    rstd = smp.tile([P, NC], f32, tag="rstd", bufs=2, name="rstd")
    nc.scalar.activation(rstd[:], mv[:, :, 1], Act.Sqrt, bias=1e-5)
    nc.vector.reciprocal(rstd[:], rstd[:])
    for c in range(NC):
        o = out_t[:, c, :]
        nc.vector.tensor_scalar(
            o, z_t[:, c, :], mv[:, c, 0:1], rstd[:, c:c + 1],
            AluOp.subtract, AluOp.mult,
        )
        nc.vector.tensor_tensor(o, o, g_b, AluOp.mult)
        nc.vector.tensor_tensor(o, o, b_b, AluOp.add)


# ---------------------------------------------------------------------------
# Host-side staging
# ---------------------------------------------------------------------------

def _pos_embedding():
    pos = np.arange(L, dtype=np.float32)[:, None]
    div = np.exp(np.arange(0, D, 2, dtype=np.float32) * -(math.log(10000.0) / D))
    pe = np.zeros((L, D), dtype=np.float32)
    pe[:, 0::2] = np.sin(pos * div)
    pe[:, 1::2] = np.cos(pos * div)
    return pe


def _bcast(v):
    return np.broadcast_to(np.asarray(v, np.float32)[None, :], (P, v.shape[-1])).copy()


def _stage_shared(p):
    """Staging that is identical for every core (computed once)."""
    import ml_dtypes

    bf16 = ml_dtypes.bfloat16

    c1b = np.asarray(p['conv1_b'], np.float32)  # (NL, DFF)
    c1bt = np.zeros((P, NL, NK), np.float32)
    for l in range(NL):
        c1bt[:, l, :] = c1b[l].reshape(NK, P).T

    # conv1: (NL, D, DFF) -> [l, kd, p, k, m]
    conv1 = np.asarray(p['conv1_w'], np.float32).reshape(NL, ND, P, NK, P)
    conv1 = np.ascontiguousarray(conv1.transpose(0, 3, 2, 1, 4)).astype(bf16)
    # conv2: (NL, DFF, D) -> [l, kd, p, n]
    conv2 = np.ascontiguousarray(
        np.asarray(p['conv2_w'], np.float32).reshape(NL, NK, P, D)
    ).astype(bf16)

    lnf = np.zeros((P, 2, D), np.float32)
    lnf[:, 0, :] = _bcast(np.asarray(p['lnf_g'], np.float32))
    lnf[:, 1, :] = _bcast(np.asarray(p['lnf_b'], np.float32))

    projw = np.asarray(p['proj_w'], np.float32).reshape(ND, P, CIN)
    projw = np.ascontiguousarray(projw.transpose(1, 0, 2)).astype(bf16)

    idx = np.arange(L, dtype=np.float32)
    dist2 = ((idx[:, None] - idx[None, :]) ** 2).reshape(NC, P, L)
    dist2 = np.ascontiguousarray(dist2.transpose(1, 0, 2))

    pos = _pos_embedding().reshape(NC, P, D)
    pos = np.ascontiguousarray(pos.transpose(1, 0, 2))

    return {
        'tokw': np.ascontiguousarray(
            np.asarray(p['tok_w'], np.float32).reshape(3 * CIN, D)
        ),
        'pos': pos,
        'dist2': dist2,
        'conv1': conv1,
        'c1b': c1bt,
        'conv2': conv2,
        'lnf': lnf,
        'projw': projw,
        'projb': _bcast(np.asarray(p['proj_b'], np.float32)),
    }


def _stage_core(x, p, b, half, shared):
    import ml_dtypes

    bf16 = ml_dtypes.bfloat16

    xb = np.asarray(x[b], dtype=np.float32)
    xcat = np.concatenate(
        [np.roll(xb, 1, axis=0), xb, np.roll(xb, -1, axis=0)], axis=1
    )

    def permc(w):
        w = np.asarray(w, np.float32)
        wh = w.reshape(w.shape[:-1] + (H, w.shape[-1] // H))
        if half:
            wh = np.concatenate([wh[..., HH:, :], wh[..., :HH, :]], axis=-2)
        return np.ascontiguousarray(wh.reshape(w.shape))

    # wq/wk: (NL, D, D) -> [l, p, hp, k, m]   (k = contraction d-tile)
    wq = permc(p['Wq']).reshape(NL, ND, P, ND, P)
    wq = np.ascontiguousarray(wq.transpose(0, 2, 3, 1, 4)).astype(bf16)
    wk = permc(p['Wk']).reshape(NL, ND, P, ND, P)
    wk = np.ascontiguousarray(wk.transpose(0, 2, 3, 1, 4)).astype(bf16)
    wv = permc(p['Wv']).reshape(NL, ND, P, D)
    wv = np.ascontiguousarray(wv.transpose(0, 2, 1, 3)).astype(bf16)

    wsig = np.asarray(p['Wsig'], np.float32)
    bsig = np.asarray(p['bsig'], np.float32)
    if half:
        wsig, bsig = wsig[:, :, HH:], bsig[:, HH:]
    else:
        wsig, bsig = wsig[:, :, :HH], bsig[:, :HH]
    wsig = wsig.reshape(NL, ND, P, HH)
    wsig = np.ascontiguousarray(wsig.transpose(0, 2, 1, 3))

    # wo rows permuted: (NL, D, D) -> [l, p, hp, n]
    wo = np.asarray(p['Wo'], np.float32)
    woh = wo.reshape(NL, H, E, D)
    if half:
        woh = np.concatenate([woh[:, HH:], woh[:, :HH]], axis=1)
    wo = woh.reshape(NL, ND, P, D)
    wo = np.ascontiguousarray(wo.transpose(0, 2, 1, 3)).astype(bf16)

    bq = permc(p['bq'])
    bk = permc(p['bk'])
    bqk = np.zeros((P, NL, 2, ND), np.float32)
    for l in range(NL):
        for hp in range(ND):
            bqk[:, l, 0, hp] = bq[l, hp * P:(hp + 1) * P]
            bqk[:, l, 1, hp] = bk[l, hp * P:(hp + 1) * P]

    bv0 = np.asarray(p['bv'], np.float32)
    wo0 = np.asarray(p['Wo'], np.float32)
    bo_eff = np.asarray(p['bo'], np.float32) + np.einsum('ld,ldn->ln', bv0, wo0)
    bc512 = np.zeros((NL, P, 7, D), np.float32)
    for l in range(NL):
        for i, vec in enumerate([
            np.zeros(D, np.float32), bo_eff[l],
            np.asarray(p['conv2_b'], np.float32)[l],
            np.asarray(p['ln1_g'], np.float32)[l], np.asarray(p['ln1_b'], np.float32)[l],
            np.asarray(p['ln2_g'], np.float32)[l], np.asarray(p['ln2_b'], np.float32)[l],
        ]):
            bc512[l, :, i, :] = np.broadcast_to(vec[None, :], (P, D))

    bsigb = np.zeros((P, NL, HH), np.float32)
    for l in range(NL):
        bsigb[:, l, :] = np.broadcast_to(bsig[l][None, :], (P, HH))

    return {
        'xcatT': np.ascontiguousarray(xcat.T),
        'tokw': shared['tokw'],
        'pos': shared['pos'],
        'dist2': shared['dist2'],
        'wq': wq, 'wk': wk, 'wv': wv,
        'bqk': bqk,
        'wsig': wsig, 'bsigb': bsigb,
        'wo': wo,
        'conv1': shared['conv1'],
        'c1b': shared['c1b'],
        'conv2': shared['conv2'],
        'bc512': bc512,
        'lnf': shared['lnf'],
        'projw': shared['projw'],
        'projb': shared['projb'],
    }


def kernel(x=None, params=None, **kw):
    from concourse.bass_utils import run_bass_kernel_spmd

    _patch_drain_wait_split()
    if 'nc' not in _CACHE:
        _CACHE['nc'] = _build_nc()
    nc = _CACHE['nc']

    x = np.asarray(x, np.float32)
    p = {k: np.asarray(v) for k, v in params.items()}

    shared = _stage_shared(p)
    in_maps = [_stage_core(x, p, c // 2, c % 2, shared) for c in range(8)]

    res = run_bass_kernel_spmd(nc, in_maps, list(range(8)))
    results = res.results

    out = np.zeros((B, L, CIN), np.float32)
    series = np.zeros((NL, B, H, L, L), np.float32)
    prior = np.zeros((NL, B, H, L, L), np.float32)
    for core in range(8):
        b, half = core // 2, core % 2
        rr = results[core]
        if half == 0:
            out[b] = rr["out_o"]
        series[:, b, half * HH:(half + 1) * HH] = rr["series_o"]
        prior[:, b, half * HH:(half + 1) * HH] = rr["prior_o"]
    return out, series, prior
